# revision 34
# baseline (speedup 1.0000x reference)
"""PointerNet fused kernel for Trainium2 (8 NeuronCores, Bass/Tile).

Problem (B=4, T=256, I=512, H=8, V=32000, D=512):
    attn    = mean_h(attn_heads)                          (B,T,I)
    context = attn @ enc_output                           (B,T,D)
    p_gen   = sigmoid([context|dec|tar] @ w + b)          (B,T,1)
    pointer = log_softmax(scatter_add(attn -> vocab))     (B,T,V)
    pointer = sw * pointer + sb
    final   = p_gen * generator_output + (1-p_gen) * pointer
    returns (final, pointer, squeeze(p_gen))

Key observation: the scattered tensor is zero outside the <=512 distinct
token columns per batch, so each log_softmax row is a constant (-lse)
except at token columns, and lse needs only a 512-slot logsumexp.  The
dense scatter is never materialized in DRAM:

  * Host builds, per batch, a duplicate-merge one-hot M (I x 512) that
    maps input positions to compact distinct-token slots, plus an int16
    gather-index map idx (per vocab position: 1+slot if it is a token of
    this batch else 0, in the GpSimd ap_gather 16-partition-wrapped
    layout).
  * Device computes g = attn @ M once (slot values, duplicates merged),
    prepends a zero column, takes exp+accum for the logsumexp, and then
    for each 1024-wide vocab chunk expands g into the dense scatter with
    ONE GpSimd ap_gather (exact fp32, no matmul), fused with the
    log_softmax affine and the p_gen blend while generator_output
    streams through SBUF.

Per-core traffic ~55MB (read gen 16.4MB + heads 2.1 + misc, write
fin/ptr 32.8MB) -> memory-roofline bound.

Sharding: (B,T) rows flattened to 1024 rows, 128 rows/core; core c
handles batch c//2, t-range (c%2)*128..+128.  No collectives needed.
"""

import numpy as np

import concourse.bass as bass
import concourse.tile as tile
from concourse import bacc, mybir
from concourse.bass_utils import run_bass_kernel_spmd

F32 = mybir.dt.float32
I16 = mybir.dt.int16
AF = mybir.ActivationFunctionType
ALU = mybir.AluOpType

B, T, I, H, V, D = 4, 256, 512, 8, 32000, 512
NCORES = 8
CORES_PER_B = NCORES // B          # 2
TC = T // CORES_PER_B              # 128 rows per core
CHUNK = 1024
NCHUNK = (V + CHUNK - 1) // CHUNK  # 32 (last chunk width 256)
LASTW = V - (NCHUNK - 1) * CHUNK
SPADC = 512                        # compact slot space (distinct tokens <= I)
GELEMS = 1 + SPADC                 # gather source: zero column + slots
IDXW = CHUNK // 16                 # idx columns per chunk in wrapped layout


def build_schedule(tokens: np.ndarray):
    """tokens: (B, I) int. Returns (M[B] (I,SPADC) f32, IDX[B] (128, NCHUNK*IDXW) i16)."""
    Ms, IDXs = [], []
    for b in range(B):
        u = np.unique(tokens[b])
        assert len(u) <= SPADC
        slot = {t: s for s, t in enumerate(u.tolist())}
        Mb = np.zeros((I, SPADC), np.float32)
        Mb[np.arange(I), [slot[t] for t in tokens[b].tolist()]] = 1.0
        idxfull = np.zeros(V, np.int16)
        idxfull[u] = 1 + np.arange(len(u), dtype=np.int16)
        idx_all = np.zeros((128, NCHUNK * IDXW), np.int16)
        for n in range(NCHUNK):
            w = CHUNK if n < NCHUNK - 1 else LASTW
            blk = idxfull[n * CHUNK:n * CHUNK + w].reshape(w // 16, 16).T
            idx_all[:, n * IDXW:n * IDXW + w // 16] = np.tile(blk, (8, 1))
        Ms.append(Mb)
        IDXs.append(idx_all)
    return Ms, IDXs


def build_program(sw: float, sb: float, pgb: float,
                  nchunk: int = NCHUNK, do_scat: bool = True):
    nc = bacc.Bacc()
    heads_d = nc.dram_tensor("heads", (H, TC, I), F32, kind="ExternalInput")
    enc_d = nc.dram_tensor("enc", (I, D), F32, kind="ExternalInput")
    dec_d = nc.dram_tensor("dec", (TC, D), F32, kind="ExternalInput")
    tar_d = nc.dram_tensor("tar", (TC, D), F32, kind="ExternalInput")
    gen_d = nc.dram_tensor("gen", (TC, V), F32, kind="ExternalInput")
    m_d = nc.dram_tensor("m_mat", (I, SPADC), F32, kind="ExternalInput")
    idx_d = nc.dram_tensor("idx_mat", (128, NCHUNK * IDXW), I16,
                           kind="ExternalInput")
    w_d = nc.dram_tensor("wrep", (3, TC, D), F32, kind="ExternalInput")
    id8_d = nc.dram_tensor("id8", (128, 128), F32, kind="ExternalInput")
    fin_d = nc.dram_tensor("fin", (TC, V), F32, kind="ExternalOutput")
    ptr_d = nc.dram_tensor("ptr", (TC, V), F32, kind="ExternalOutput")
    pg_d = nc.dram_tensor("pg", (TC, 1), F32, kind="ExternalOutput")

    with tile.TileContext(nc) as tc:
        with (
            tc.tile_pool(name="const", bufs=1) as cpool,
            tc.tile_pool(name="io_in", bufs=6) as ipool,
            tc.tile_pool(name="io_out", bufs=4) as opool,
        ):
          with (
            tc.tile_pool(name="hload", bufs=1) as hpool,
            tc.tile_pool(name="work", bufs=2) as wpool,
            tc.tile_pool(name="ps2", bufs=2, space="PSUM") as ps2,
            tc.tile_pool(name="psctx", bufs=1, space="PSUM") as psctx,
            tc.tile_pool(name="psg", bufs=1, space="PSUM") as psg,
          ):
            # ---------- constant / prologue loads ----------
            id8 = cpool.tile([128, 128], F32, tag="id8")
            nc.sync.dma_start(id8[:], id8_d[:])
            enc_t = []
            for j in range(4):
                e = cpool.tile([128, D], F32, tag=f"enc{j}")
                nc.sync.dma_start(e[:], enc_d[j * 128:(j + 1) * 128, :])
                enc_t.append(e)
            m_t = []
            for j in range(4):
                m = hpool.tile([128, SPADC], F32, tag=f"m{j}")
                nc.sync.dma_start(m[:], m_d[j * 128:(j + 1) * 128, :])
                m_t.append(m)
            idx_t = cpool.tile([128, NCHUNK * IDXW], I16, tag="idx")
            nc.sync.dma_start(idx_t[:], idx_d[:])
            wr = []
            for k in range(3):
                w = cpool.tile([TC, D], F32, tag=f"w{k}")
                nc.sync.dma_start(w[:], w_d[k])
                wr.append(w)
            dec_t = cpool.tile([TC, D], F32, tag="dec")
            nc.sync.dma_start(dec_t[:], dec_d[:])
            tar_t = cpool.tile([TC, D], F32, tag="tar")
            nc.sync.dma_start(tar_t[:], tar_d[:])
            vbias = cpool.tile([128, 1], F32, tag="vbias")
            nc.vector.memset(vbias[:], float(V - SPADC))
            pgb_t = cpool.tile([128, 1], F32, tag="pgb")
            nc.vector.memset(pgb_t[:], pgb)
            npgb_t = cpool.tile([128, 1], F32, tag="npgb")
            nc.vector.memset(npgb_t[:], -pgb)

            # ---------- attn head-mean + transpose (via Id/8 matmul) ----------
            hts = []
            for h in range(H):
                ht = hpool.tile([TC, I], F32, tag=f"h{h}")
                nc.sync.dma_start(ht[:], heads_d[h])
                hts.append(ht)
            stride = 1
            while stride < H:
                for k in range(0, H, 2 * stride):
                    nc.vector.tensor_add(hts[k][:], hts[k][:],
                                         hts[k + stride][:])
                stride *= 2
            ssum = hts[0]  # (t, i) sum of heads

            aT = []  # attn.T / 8 tiles: (i-part 128, t 128)
            for j in range(4):
                ps = ps2.tile([128, 128], F32, tag="psA")
                nc.tensor.matmul(ps[:], lhsT=ssum[:, j * 128:(j + 1) * 128],
                                 rhs=id8[:], start=True, stop=True)
                a = cpool.tile([128, 128], F32, tag=f"aT{j}")
                nc.vector.tensor_copy(a[:], ps[:])
                aT.append(a)

            # ---------- context = attn @ enc  (t-part, d) ----------
            ctxps = psctx.tile([128, D], F32, tag="ctx")
            for j in range(4):
                nc.tensor.matmul(ctxps[:], lhsT=aT[j][:], rhs=enc_t[j][:],
                                 start=(j == 0), stop=(j == 3))

            # ---------- g = attn @ M  (t-part, slot) + zero column ----------
            gps = psg.tile([128, SPADC], F32, tag="g")
            for j in range(4):
                nc.tensor.matmul(gps[:], lhsT=aT[j][:], rhs=m_t[j][:],
                                 start=(j == 0), stop=(j == 3))
            gflat = cpool.tile([128, GELEMS], F32, tag="gflat")
            nc.vector.memset(gflat[:, 0:1], 0.0)
            nc.vector.tensor_copy(gflat[:, 1:GELEMS], gps[:])

            # ---------- lse = log((V - SPADC) + sum_s exp(g)) per t ----------
            escr = wpool.tile([128, SPADC], F32, tag="escr")
            seps = cpool.tile([128, 1], F32, tag="seps")
            nc.scalar.activation(escr[:], gps[:], AF.Exp, accum_out=seps[:])
            lse = cpool.tile([128, 1], F32, tag="lse")
            nc.scalar.activation(lse[:], seps[:], AF.Ln, bias=vbias[:])

            # ---------- p_gen ----------
            zc = cpool.tile([128, 1], F32, tag="zc")
            zd = cpool.tile([128, 1], F32, tag="zd")
            zt = cpool.tile([128, 1], F32, tag="zt")
            for src, wk, acc in ((ctxps, wr[0], zc), (dec_t, wr[1], zd),
                                 (tar_t, wr[2], zt)):
                scr = wpool.tile([TC, D], F32, tag="pscr")
                nc.vector.tensor_mul(scr[:], src[:], wk[:])
                nc.vector.tensor_reduce(acc[:], scr[:],
                                        axis=mybir.AxisListType.X, op=ALU.add)
            z1 = cpool.tile([128, 1], F32, tag="z1")
            nc.vector.tensor_add(z1[:], zc[:], zd[:])
            z = cpool.tile([128, 1], F32, tag="z")
            nc.vector.tensor_add(z[:], z1[:], zt[:])
            pg = cpool.tile([128, 1], F32, tag="pg")
            nc.scalar.activation(pg[:], z[:], AF.Sigmoid, bias=pgb_t[:])
            q = cpool.tile([128, 1], F32, tag="q")
            nc.scalar.activation(q[:], z[:], AF.Sigmoid, scale=-1.0,
                                 bias=npgb_t[:])
            nc.sync.dma_start(pg_d[:], pg[:])

            # r2 = sb - sw*lse ; r = q*r2 ; qsw = q*sw
            r2 = cpool.tile([128, 1], F32, tag="r2")
            nc.scalar.activation(r2[:], lse[:], AF.Copy, scale=-sw, bias=sb)
            r = cpool.tile([128, 1], F32, tag="r")
            nc.vector.tensor_mul(r[:], q[:], r2[:])
            qsw = cpool.tile([128, 1], F32, tag="qsw")
            nc.vector.tensor_scalar_mul(qsw[:], q[:], sw)

          # ---------- streamed vocab chunks (prologue pools closed) ----------
          for n in range(nchunk):
                w = CHUNK if n < NCHUNK - 1 else LASTW
                off = n * CHUNK

                gent = ipool.tile([TC, CHUNK], F32, tag="gen")
                nc.sync.dma_start(gent[:, :w], gen_d[:, off:off + w])

                ptrt = opool.tile([TC, CHUNK], F32, tag="ptr")
                fint = opool.tile([TC, CHUNK], F32, tag="fin")
                if do_scat:
                    scat = opool.tile([TC, CHUNK], F32, tag="scat")
                    nc.gpsimd.ap_gather(
                        scat[:, :w], gflat[:],
                        idx_t[:, n * IDXW:n * IDXW + w // 16],
                        channels=128, num_elems=GELEMS, d=1, num_idxs=w)
                    # ptr = sw*scat + r2 ; tmp = qsw*scat + r ; fin = pg*gen+tmp
                    nc.vector.tensor_scalar(
                        ptrt[:, :w], scat[:, :w], sw, r2[:],
                        op0=ALU.mult, op1=ALU.add)
                    tmpt = opool.tile([TC, CHUNK], F32, tag="tmp")
                    nc.scalar.activation(tmpt[:, :w], scat[:, :w], AF.Identity,
                                         scale=qsw[:], bias=r[:])
                    nc.vector.scalar_tensor_tensor(
                        fint[:, :w], in0=gent[:, :w], scalar=pg[:],
                        in1=tmpt[:, :w], op0=ALU.mult, op1=ALU.add)
                else:
                    # background only: ptr = r2 ; fin = pg*gen + r
                    nc.scalar.activation(ptrt[:, :w], gent[:, :w], AF.Identity,
                                         scale=0.0, bias=r2[:])
                    nc.scalar.activation(fint[:, :w], gent[:, :w], AF.Identity,
                                         scale=pg[:], bias=r[:])

                nc.scalar.dma_start(fin_d[:, off:off + w], fint[:, :w])
                nc.scalar.dma_start(ptr_d[:, off:off + w], ptrt[:, :w])
    return nc


def _in_maps(inputs, Ms, IDXs):
    heads = np.asarray(inputs["attn_heads"], np.float32)
    enc = np.asarray(inputs["enc_output"], np.float32)
    dec = np.asarray(inputs["dec_state"], np.float32)
    tar = np.asarray(inputs["tar_embedded"], np.float32)
    gen = np.asarray(inputs["generator_output"], np.float32)
    wfull = np.asarray(inputs["p_gen_w"], np.float32).reshape(3, D)
    wrep = np.broadcast_to(wfull[:, None, :], (3, TC, D)).copy()
    id8 = (np.eye(128, dtype=np.float32) / H).copy()
    maps = []
    for c in range(NCORES):
        b, t0 = c // CORES_PER_B, (c % CORES_PER_B) * TC
        maps.append({
            "heads": np.ascontiguousarray(heads[b, :, t0:t0 + TC, :]),
            "enc": np.ascontiguousarray(enc[b]),
            "dec": np.ascontiguousarray(dec[b, t0:t0 + TC]),
            "tar": np.ascontiguousarray(tar[b, t0:t0 + TC]),
            "gen": np.ascontiguousarray(gen[b, t0:t0 + TC]),
            "m_mat": Ms[b],
            "idx_mat": IDXs[b],
            "wrep": wrep,
            "id8": id8,
        })
    return maps


def _run(inputs, trace=False):
    tokens = np.asarray(inputs["inp_tokens"])
    Ms, IDXs = build_schedule(tokens)
    sw = float(np.asarray(inputs["pointer_scale_w"]))
    sb = float(np.asarray(inputs["pointer_scale_b"]))
    pgb = float(np.asarray(inputs["p_gen_b"]).reshape(-1)[0])
    nc = build_program(sw, sb, pgb)
    nc.finalize()
    maps = _in_maps(inputs, Ms, IDXs)
    res = run_bass_kernel_spmd(nc, maps, list(range(NCORES)), trace=trace)
    fin = np.empty((B, T, V), np.float32)
    ptr = np.empty((B, T, V), np.float32)
    pgo = np.empty((B, T), np.float32)
    for c in range(NCORES):
        b, t0 = c // CORES_PER_B, (c % CORES_PER_B) * TC
        fin[b, t0:t0 + TC] = res.results[c]["fin"]
        ptr[b, t0:t0 + TC] = res.results[c]["ptr"]
        pgo[b, t0:t0 + TC] = res.results[c]["pg"][:, 0]
    return (fin, ptr, pgo), res


def kernel(**inputs):
    out, _ = _run(inputs, trace=False)
    return out


def _install_profile_hook():
    """Provide antenv.axon_hooks (missing in this container) so
    run_bass_kernel_spmd(trace=True) can reach libaxon's NRT profiler."""
    import contextlib
    import ctypes
    import sys
    import types
    try:
        import antenv.axon_hooks  # noqa: F401
        return
    except ImportError:
        pass

    so_path = "/opt/axon/libaxon_pjrt.so"
    lib = ctypes.CDLL(so_path)
    hook = None
    if hasattr(lib, "axon_start_nrt_profile"):
        lib.axon_start_nrt_profile.argtypes = [
            ctypes.POINTER(ctypes.c_int64), ctypes.c_size_t]
        lib.axon_start_nrt_profile.restype = ctypes.c_int64
        lib.axon_stop_nrt_profile.argtypes = [ctypes.c_char_p]
        lib.axon_stop_nrt_profile.restype = ctypes.c_int64

        @contextlib.contextmanager
        def hook(output_dir, device_ids):
            import jax
            jax.devices()
            if device_ids:
                ids = (ctypes.c_int64 * len(device_ids))(*device_ids)
                rc = lib.axon_start_nrt_profile(ids, len(device_ids))
            else:
                rc = lib.axon_start_nrt_profile(None, 0)
            if rc != 0:
                raise RuntimeError(f"axon_start_nrt_profile rc={rc}")
            try:
                yield
            finally:
                n = lib.axon_stop_nrt_profile(str(output_dir).encode())
                print(f"profile: {n} ntff file(s) -> {output_dir}",
                      file=sys.stderr)

    mod = types.ModuleType("antenv.axon_hooks")
    mod.get_axon_ntff_profile_hook = lambda: hook
    mod.set_axon_ntff_profile_hook = lambda h: None
    sys.modules["antenv.axon_hooks"] = mod


def kernel_profiled(**inputs):
    _install_profile_hook()
    import concourse.bass_utils as bu
    bu.upload_artifacts = lambda tmpdir: str(tmpdir)  # no bucket in container
    out, res = _run(inputs, trace=True)
    return out, res


# revision 43
# speedup vs baseline: 4.5652x; 4.5652x over previous
"""PointerNet fused kernel for Trainium2 (8 NeuronCores, Bass/Tile).

Problem (B=4, T=256, I=512, H=8, V=32000, D=512):
    attn    = mean_h(attn_heads)                          (B,T,I)
    context = attn @ enc_output                           (B,T,D)
    p_gen   = sigmoid([context|dec|tar] @ w + b)          (B,T,1)
    pointer = log_softmax(scatter_add(attn -> vocab))     (B,T,V)
    pointer = sw * pointer + sb
    final   = p_gen * generator_output + (1-p_gen) * pointer
    returns (final, pointer, squeeze(p_gen))

Key observation: the scattered tensor is zero outside the <=512 token
columns per batch, so log_softmax rows are a per-row constant
(-lse, from a 512-slot logsumexp) except at token columns.  We never
materialize the dense scatter in DRAM:

  * Host builds, per batch, a duplicate-merge one-hot M (I x SPAD) that
    maps input positions to distinct-token "slots" (sorted by token id,
    grouped so chunk n of the vocab owns slot range [KB*n, KB*n+KB)),
    and a placement one-hot P (SPAD x 512) mapping slot -> column-within-
    chunk.  KB is a divisor of 128 so slot ranges never straddle SBUF
    partition tiles, giving a single uniform SPMD program for all cores.
  * Device computes gT = M.T @ attn.T (slots x T) once, then for each
    512-wide vocab chunk does ONE tiny matmul (K=KB) producing the dense
    scatter chunk directly in PSUM, fused with the log_softmax affine and
    the p_gen blend while streaming generator_output through SBUF.

Per-core traffic ~58MB (read gen 16.4MB + heads 2.1 + misc, write
fin/ptr 32.8MB) -> memory-roofline bound.

Sharding: (B,T) rows flattened to 1024 rows, 128 rows/core; core c
handles batch c//2, t-range (c%2)*128..+128.  No collectives needed.
"""

import numpy as np

import concourse.bass as bass
import concourse.tile as tile
from concourse import bacc, mybir
from concourse.bass_utils import run_bass_kernel_spmd

F32 = mybir.dt.float32
F32R = mybir.dt.float32r  # single-pass fp32 matmul mode (4x faster at N>=256)
AF = mybir.ActivationFunctionType
ALU = mybir.AluOpType

B, T, I, H, V, D = 4, 256, 512, 8, 32000, 512
NCORES = 8
CORES_PER_B = NCORES // B          # 2
TC = T // CORES_PER_B              # 128 rows per core
CHUNK = 1024
NCHUNK = (V + CHUNK - 1) // CHUNK  # 32 (last chunk width 256)
LASTW = V - (NCHUNK - 1) * CHUNK
MMW = 512                          # max matmul moving free dim / PSUM bank


def _pick_kb(max_per_chunk: int) -> int:
    # PE matmul operands must start at partition 0/32/64 (not 96) -> pack
    # chunks per 128-partition tile at those bases only.
    for kb in (32, 64, 128):
        if kb >= max_per_chunk:
            return kb
    raise ValueError(f"too many distinct tokens in one vocab chunk: {max_per_chunk}")


def _chunk_slot_base(n: int, KB: int) -> tuple[int, int]:
    """Return (tile index sj, partition base p0) for vocab chunk n."""
    cpt = {32: 3, 64: 2, 128: 1}[KB]  # chunks per 128-partition tile
    return n // cpt, KB * (n % cpt)


def build_schedule(tokens: np.ndarray):
    """tokens: (B, I) int. Returns (KB, SPAD, M[B], P[B])."""
    uniq = [np.unique(tokens[b]) for b in range(B)]
    mx = 1
    for u in uniq:
        cnt = np.bincount(u // CHUNK, minlength=NCHUNK)
        mx = max(mx, int(cnt.max()))
    KB = _pick_kb(mx)
    cpt = {32: 3, 64: 2, 128: 1}[KB]
    SPAD = ((NCHUNK + cpt - 1) // cpt) * 128
    Ms, Ps = [], []
    for b in range(B):
        u = uniq[b]
        chunks = u // CHUNK
        rank = np.arange(len(u)) - np.searchsorted(chunks, chunks, side="left")
        sj, p0 = np.vectorize(lambda n: _chunk_slot_base(n, KB))(chunks)
        slots = 128 * sj + p0 + rank
        assert slots.max() < SPAD and rank.max() < KB
        Mb = np.zeros((I, SPAD), np.float32)
        tok2slot = dict(zip(u.tolist(), slots.tolist()))
        slot_of_pos = np.array([tok2slot[t] for t in tokens[b].tolist()])
        Mb[np.arange(I), slot_of_pos] = 1.0
        Pb = np.zeros((SPAD, CHUNK), np.float32)
        Pb[slots, u % CHUNK] = 1.0
        Ms.append(Mb)
        Ps.append(Pb)
    return KB, SPAD, Ms, Ps


def build_program(KB: int, SPAD: int, sw: float, sb: float, pgb: float,
                  nchunk: int = NCHUNK, do_scat: bool = True, stage: int = 99):
    nc = bacc.Bacc()
    heads_d = nc.dram_tensor("heads", (H, TC, I), F32, kind="ExternalInput")
    enc_d = nc.dram_tensor("enc", (I, D), F32, kind="ExternalInput")
    dec_d = nc.dram_tensor("dec", (TC, D), F32, kind="ExternalInput")
    tar_d = nc.dram_tensor("tar", (TC, D), F32, kind="ExternalInput")
    gen_d = nc.dram_tensor("gen", (TC, V), F32, kind="ExternalInput")
    m_d = nc.dram_tensor("m_mat", (I, SPAD), F32R, kind="ExternalInput")
    p_d = nc.dram_tensor("p_mat", (SPAD, CHUNK), F32R, kind="ExternalInput")
    w_d = nc.dram_tensor("wrep", (3, TC, D), F32, kind="ExternalInput")
    id8_d = nc.dram_tensor("id8", (128, 128), F32, kind="ExternalInput")
    fin_d = nc.dram_tensor("fin", (TC, V), F32, kind="ExternalOutput")
    ptr_d = nc.dram_tensor("ptr", (TC, V), F32, kind="ExternalOutput")
    pg_d = nc.dram_tensor("pg", (TC, 1), F32, kind="ExternalOutput")

    NSJ = SPAD // 128

    with tile.TileContext(nc) as tc:
        with (
            tc.tile_pool(name="const", bufs=1) as cpool,
            tc.tile_pool(name="io_in", bufs=6) as ipool,
            tc.tile_pool(name="io_out", bufs=4) as opool,
        ):
          with (
            tc.tile_pool(name="hload", bufs=1) as hpool,
            tc.tile_pool(name="mload", bufs=1) as mpool,
            tc.tile_pool(name="work", bufs=2) as wpool,
            tc.tile_pool(name="ps2", bufs=2, space="PSUM") as ps2,
            tc.tile_pool(name="psctx", bufs=1, space="PSUM") as psctx,
          ):
            # ---------- constant / prologue loads ----------
            id8 = cpool.tile([128, 128], F32, tag="id8")
            nc.sync.dma_start(id8[:], id8_d[:])
            enc_t = []
            for j in range(4):
                e = cpool.tile([128, D], F32, tag=f"enc{j}")
                nc.sync.dma_start(e[:], enc_d[j * 128:(j + 1) * 128, :])
                enc_t.append(e)
            m_t = []
            for j in range(4):
                m = mpool.tile([128, SPAD], F32R, tag=f"m{j}")
                nc.sync.dma_start(m[:], m_d[j * 128:(j + 1) * 128, :])
                m_t.append(m)
            p_t = []
            for sj in range(NSJ):
                p = cpool.tile([128, CHUNK], F32R, tag=f"p{sj}")
                nc.sync.dma_start(p[:], p_d[sj * 128:(sj + 1) * 128, :])
                p_t.append(p)
            wr = []
            for k in range(3):
                w = cpool.tile([TC, D], F32, tag=f"w{k}")
                nc.sync.dma_start(w[:], w_d[k])
                wr.append(w)
            dec_t = cpool.tile([TC, D], F32, tag="dec")
            nc.sync.dma_start(dec_t[:], dec_d[:])
            tar_t = cpool.tile([TC, D], F32, tag="tar")
            nc.sync.dma_start(tar_t[:], tar_d[:])
            ones = cpool.tile([128, 1], F32, tag="ones")
            nc.vector.memset(ones[:], 1.0)
            vbias = cpool.tile([128, 1], F32, tag="vbias")
            nc.vector.memset(vbias[:], float(V - SPAD))
            pgb_t = cpool.tile([128, 1], F32, tag="pgb")
            nc.vector.memset(pgb_t[:], pgb)
            npgb_t = cpool.tile([128, 1], F32, tag="npgb")
            nc.vector.memset(npgb_t[:], -pgb)

            # ---------- attn head-mean + transpose (via Id/8 matmul) ----------
            hts = []
            for h in range(H):
                ht = hpool.tile([TC, I], F32, tag=f"h{h}")
                nc.sync.dma_start(ht[:], heads_d[h])
                hts.append(ht)
            # pairwise in-place tree sum of the 8 heads
            stride = 1
            while stride < H:
                for k in range(0, H, 2 * stride):
                    nc.vector.tensor_add(hts[k][:], hts[k][:],
                                         hts[k + stride][:])
                stride *= 2
            ssum = hts[0]  # (t, i) sum of heads
            if stage == 1:
                nc.sync.dma_start(pg_d[:], ssum[:, 0:1])
                return nc

            aT = []   # attn.T / 8 tiles: (i-part 128, t 128), fp32 for ctx
            aTr = []  # f32r-rounded copies for the f32r g matmuls
            for j in range(4):
                ps = ps2.tile([128, 128], F32, tag="psA")
                nc.tensor.matmul(ps[:], lhsT=ssum[:, j * 128:(j + 1) * 128],
                                 rhs=id8[:], start=True, stop=True)
                a = cpool.tile([128, 128], F32, tag=f"aT{j}")
                nc.vector.tensor_copy(a[:], ps[:])
                aT.append(a)
                ar = cpool.tile([128, 128], F32R, tag=f"aTr{j}")
                nc.vector.tensor_copy(ar[:], ps[:])
                aTr.append(ar)
            if stage == 2:
                nc.sync.dma_start(pg_d[:], aT[0][:, 0:1])
                return nc

            # ---------- context = attn @ enc  (t-part, d) ----------
            ctxps = psctx.tile([128, D], F32, tag="ctx")
            for j in range(4):
                nc.tensor.matmul(ctxps[:], lhsT=aT[j][:], rhs=enc_t[j][:],
                                 start=(j == 0), stop=(j == 3))

            # ---------- gT = M.T @ attn.T  (slot-part, t) ----------
            gt = []
            eg = []
            for sj in range(NSJ):
                gps = ps2.tile([128, 128], F32, tag="psA")
                for j in range(4):
                    nc.tensor.matmul(
                        gps[:],
                        lhsT=m_t[j][:, sj * 128:(sj + 1) * 128],
                        rhs=aTr[j][:],
                        start=(j == 0), stop=(j == 3))
                g = cpool.tile([128, 128], F32R, tag=f"gt{sj}")
                nc.vector.tensor_copy(g[:], gps[:])
                gt.append(g)
                e = mpool.tile([128, 128], F32, tag=f"eg{sj}")
                nc.scalar.activation(e[:], gps[:], AF.Exp)
                eg.append(e)
            if stage == 3:
                nc.sync.dma_start(pg_d[:], gt[0][:, 0:1])
                return nc

            # ---------- lse = log((V - SPAD) + sum_s exp(gT)) per t ----------
            seps = ps2.tile([128, 1], F32, tag="seps")
            for sj in range(NSJ):
                nc.tensor.matmul(seps[:], lhsT=eg[sj][:], rhs=ones[:],
                                 start=(sj == 0), stop=(sj == NSJ - 1))
            lse = cpool.tile([128, 1], F32, tag="lse")
            nc.scalar.activation(lse[:], seps[:], AF.Ln, bias=vbias[:])
            if stage == 4:
                nc.sync.dma_start(pg_d[:], lse[:])
                return nc

            # ---------- p_gen ----------
            zc = cpool.tile([128, 1], F32, tag="zc")
            zd = cpool.tile([128, 1], F32, tag="zd")
            zt = cpool.tile([128, 1], F32, tag="zt")
            for src, wk, acc in ((ctxps, wr[0], zc), (dec_t, wr[1], zd),
                                 (tar_t, wr[2], zt)):
                scr = wpool.tile([TC, D], F32, tag="pscr")
                nc.vector.tensor_mul(scr[:], src[:], wk[:])
                nc.vector.tensor_reduce(acc[:], scr[:],
                                        axis=mybir.AxisListType.X, op=ALU.add)
            z1 = cpool.tile([128, 1], F32, tag="z1")
            nc.vector.tensor_add(z1[:], zc[:], zd[:])
            z = cpool.tile([128, 1], F32, tag="z")
            nc.vector.tensor_add(z[:], z1[:], zt[:])
            pg = cpool.tile([128, 1], F32, tag="pg")
            nc.scalar.activation(pg[:], z[:], AF.Sigmoid, bias=pgb_t[:])
            q = cpool.tile([128, 1], F32, tag="q")
            nc.scalar.activation(q[:], z[:], AF.Sigmoid, scale=-1.0, bias=npgb_t[:])
            nc.sync.dma_start(pg_d[:], pg[:])

            # r2 = sb - sw*lse ; r = q*r2 ; qsw = q*sw
            r2 = cpool.tile([128, 1], F32, tag="r2")
            nc.scalar.activation(r2[:], lse[:], AF.Copy, scale=-sw, bias=sb)
            r = cpool.tile([128, 1], F32, tag="r")
            nc.vector.tensor_mul(r[:], q[:], r2[:])
            qsw = cpool.tile([128, 1], F32, tag="qsw")
            nc.vector.tensor_scalar_mul(qsw[:], q[:], sw)

          # ---------- streamed vocab chunks (prologue pools closed) ----------
          with tc.tile_pool(name="psscat", bufs=4, space="PSUM") as psscat:
            for n in range(nchunk):
                w = CHUNK if n < NCHUNK - 1 else LASTW
                off = n * CHUNK
                sj, p0 = _chunk_slot_base(n, KB)

                gent = ipool.tile([TC, CHUNK], F32, tag="gen")
                nc.sync.dma_start(gent[:, :w], gen_d[:, off:off + w])

                ptrt = opool.tile([TC, CHUNK], F32, tag="ptr")
                fint = opool.tile([TC, CHUNK], F32, tag="fin")
                if do_scat:
                    scat = psscat.tile([128, CHUNK], F32, tag="scat")
                    for c0 in range(0, w, MMW):
                        cw = min(MMW, w - c0)
                        nc.tensor.matmul(
                            scat[:, c0:c0 + cw],
                            lhsT=gt[sj][p0:p0 + KB, :],
                            rhs=p_t[sj][p0:p0 + KB, c0:c0 + cw],
                            start=True, stop=True)
                    # ptr = sw*scat + r2 ; tmp = qsw*scat + r ; fin = pg*gen+tmp
                    nc.vector.tensor_scalar(
                        ptrt[:, :w], scat[:, :w], sw, r2[:],
                        op0=ALU.mult, op1=ALU.add)
                    tmpt = opool.tile([TC, CHUNK], F32, tag="tmp")
                    nc.scalar.activation(tmpt[:, :w], scat[:, :w], AF.Identity,
                                         scale=qsw[:], bias=r[:])
                    nc.vector.scalar_tensor_tensor(
                        fint[:, :w], in0=gent[:, :w], scalar=pg[:],
                        in1=tmpt[:, :w], op0=ALU.mult, op1=ALU.add)
                else:
                    # background only: ptr = r2 ; fin = pg*gen + r
                    nc.scalar.activation(ptrt[:, :w], gent[:, :w], AF.Identity,
                                         scale=0.0, bias=r2[:])
                    nc.scalar.activation(fint[:, :w], gent[:, :w], AF.Identity,
                                         scale=pg[:], bias=r[:])

                nc.gpsimd.dma_start(fin_d[:, off:off + w], fint[:, :w])
                nc.gpsimd.dma_start(ptr_d[:, off:off + w], ptrt[:, :w])
    return nc


def _in_maps(inputs, Ms, Ps):
    heads = np.asarray(inputs["attn_heads"], np.float32)
    enc = np.asarray(inputs["enc_output"], np.float32)
    dec = np.asarray(inputs["dec_state"], np.float32)
    tar = np.asarray(inputs["tar_embedded"], np.float32)
    gen = np.asarray(inputs["generator_output"], np.float32)
    wfull = np.asarray(inputs["p_gen_w"], np.float32).reshape(3, D)
    wrep = np.broadcast_to(wfull[:, None, :], (3, TC, D)).copy()
    id8 = (np.eye(128, dtype=np.float32) / H).copy()
    maps = []
    for c in range(NCORES):
        b, t0 = c // CORES_PER_B, (c % CORES_PER_B) * TC
        maps.append({
            "heads": np.ascontiguousarray(heads[b, :, t0:t0 + TC, :]),
            "enc": np.ascontiguousarray(enc[b]),
            "dec": np.ascontiguousarray(dec[b, t0:t0 + TC]),
            "tar": np.ascontiguousarray(tar[b, t0:t0 + TC]),
            "gen": np.ascontiguousarray(gen[b, t0:t0 + TC]),
            "m_mat": Ms[b],
            "p_mat": Ps[b],
            "wrep": wrep,
            "id8": id8,
        })
    return maps


def _run(inputs, trace=False):
    tokens = np.asarray(inputs["inp_tokens"])
    KB, SPAD, Ms, Ps = build_schedule(tokens)
    sw = float(np.asarray(inputs["pointer_scale_w"]))
    sb = float(np.asarray(inputs["pointer_scale_b"]))
    pgb = float(np.asarray(inputs["p_gen_b"]).reshape(-1)[0])
    nc = build_program(KB, SPAD, sw, sb, pgb)
    nc.finalize()
    maps = _in_maps(inputs, Ms, Ps)
    res = run_bass_kernel_spmd(nc, maps, list(range(NCORES)), trace=trace)
    fin = np.empty((B, T, V), np.float32)
    ptr = np.empty((B, T, V), np.float32)
    pgo = np.empty((B, T), np.float32)
    for c in range(NCORES):
        b, t0 = c // CORES_PER_B, (c % CORES_PER_B) * TC
        fin[b, t0:t0 + TC] = res.results[c]["fin"]
        ptr[b, t0:t0 + TC] = res.results[c]["ptr"]
        pgo[b, t0:t0 + TC] = res.results[c]["pg"][:, 0]
    return (fin, ptr, pgo), res


def kernel(**inputs):
    out, _ = _run(inputs, trace=False)
    return out


def _install_profile_hook():
    """Provide antenv.axon_hooks (missing in this container) so
    run_bass_kernel_spmd(trace=True) can reach libaxon's NRT profiler."""
    import contextlib
    import ctypes
    import sys
    import types
    try:
        import antenv.axon_hooks  # noqa: F401
        return
    except ImportError:
        pass

    so_path = "/opt/axon/libaxon_pjrt.so"
    lib = ctypes.CDLL(so_path)
    hook = None
    if hasattr(lib, "axon_start_nrt_profile"):
        lib.axon_start_nrt_profile.argtypes = [
            ctypes.POINTER(ctypes.c_int64), ctypes.c_size_t]
        lib.axon_start_nrt_profile.restype = ctypes.c_int64
        lib.axon_stop_nrt_profile.argtypes = [ctypes.c_char_p]
        lib.axon_stop_nrt_profile.restype = ctypes.c_int64

        @contextlib.contextmanager
        def hook(output_dir, device_ids):
            import jax
            jax.devices()
            if device_ids:
                ids = (ctypes.c_int64 * len(device_ids))(*device_ids)
                rc = lib.axon_start_nrt_profile(ids, len(device_ids))
            else:
                rc = lib.axon_start_nrt_profile(None, 0)
            if rc != 0:
                raise RuntimeError(f"axon_start_nrt_profile rc={rc}")
            try:
                yield
            finally:
                n = lib.axon_stop_nrt_profile(str(output_dir).encode())
                print(f"profile: {n} ntff file(s) -> {output_dir}",
                      file=sys.stderr)

    mod = types.ModuleType("antenv.axon_hooks")
    mod.get_axon_ntff_profile_hook = lambda: hook
    mod.set_axon_ntff_profile_hook = lambda h: None
    sys.modules["antenv.axon_hooks"] = mod


def kernel_profiled(**inputs):
    _install_profile_hook()
    import concourse.bass_utils as bu
    bu.upload_artifacts = lambda tmpdir: str(tmpdir)  # no bucket in container
    out, res = _run(inputs, trace=True)
    return out, res


# revision 45
# speedup vs baseline: 4.6842x; 1.0261x over previous
"""PointerNet fused kernel for Trainium2 (8 NeuronCores, Bass/Tile).

Problem (B=4, T=256, I=512, H=8, V=32000, D=512):
    attn    = mean_h(attn_heads)                          (B,T,I)
    context = attn @ enc_output                           (B,T,D)
    p_gen   = sigmoid([context|dec|tar] @ w + b)          (B,T,1)
    pointer = log_softmax(scatter_add(attn -> vocab))     (B,T,V)
    pointer = sw * pointer + sb
    final   = p_gen * generator_output + (1-p_gen) * pointer
    returns (final, pointer, squeeze(p_gen))

Key observation: the scattered tensor is zero outside the <=512 token
columns per batch, so log_softmax rows are a per-row constant
(-lse, from a 512-slot logsumexp) except at token columns.  We never
materialize the dense scatter in DRAM:

  * Host builds, per batch, a duplicate-merge one-hot M (I x SPAD) that
    maps input positions to distinct-token "slots" (sorted by token id,
    grouped so chunk n of the vocab owns slot range [KB*n, KB*n+KB)),
    and a placement one-hot P (SPAD x 512) mapping slot -> column-within-
    chunk.  KB is a divisor of 128 so slot ranges never straddle SBUF
    partition tiles, giving a single uniform SPMD program for all cores.
  * Device computes gT = M.T @ attn.T (slots x T) once, then for each
    512-wide vocab chunk does ONE tiny matmul (K=KB) producing the dense
    scatter chunk directly in PSUM, fused with the log_softmax affine and
    the p_gen blend while streaming generator_output through SBUF.

Per-core traffic ~58MB (read gen 16.4MB + heads 2.1 + misc, write
fin/ptr 32.8MB) -> memory-roofline bound.

Sharding: (B,T) rows flattened to 1024 rows, 128 rows/core; core c
handles batch c//2, t-range (c%2)*128..+128.  No collectives needed.
"""

import numpy as np

import concourse.bass as bass
import concourse.tile as tile
from concourse import bacc, mybir
from concourse.bass_utils import run_bass_kernel_spmd

F32 = mybir.dt.float32
F32R = mybir.dt.float32r  # single-pass fp32 matmul mode (4x faster at N>=256)
AF = mybir.ActivationFunctionType
ALU = mybir.AluOpType

B, T, I, H, V, D = 4, 256, 512, 8, 32000, 512
NCORES = 8
CORES_PER_B = NCORES // B          # 2
TC = T // CORES_PER_B              # 128 rows per core
CHUNK = 1024
NCHUNK = (V + CHUNK - 1) // CHUNK  # 32 (last chunk width 256)
LASTW = V - (NCHUNK - 1) * CHUNK
MMW = 512                          # max matmul moving free dim / PSUM bank


def _pick_kb(max_per_chunk: int) -> int:
    # PE matmul operands must start at partition 0/32/64 (not 96) -> pack
    # chunks per 128-partition tile at those bases only.
    for kb in (32, 64, 128):
        if kb >= max_per_chunk:
            return kb
    raise ValueError(f"too many distinct tokens in one vocab chunk: {max_per_chunk}")


def _chunk_slot_base(n: int, KB: int) -> tuple[int, int]:
    """Return (tile index sj, partition base p0) for vocab chunk n."""
    cpt = {32: 3, 64: 2, 128: 1}[KB]  # chunks per 128-partition tile
    return n // cpt, KB * (n % cpt)


def build_schedule(tokens: np.ndarray):
    """tokens: (B, I) int. Returns (KB, SPAD, M[B], P[B])."""
    uniq = [np.unique(tokens[b]) for b in range(B)]
    mx = 1
    for u in uniq:
        cnt = np.bincount(u // CHUNK, minlength=NCHUNK)
        mx = max(mx, int(cnt.max()))
    KB = _pick_kb(mx)
    cpt = {32: 3, 64: 2, 128: 1}[KB]
    SPAD = ((NCHUNK + cpt - 1) // cpt) * 128
    Ms, Ps = [], []
    for b in range(B):
        u = uniq[b]
        chunks = u // CHUNK
        rank = np.arange(len(u)) - np.searchsorted(chunks, chunks, side="left")
        sj, p0 = np.vectorize(lambda n: _chunk_slot_base(n, KB))(chunks)
        slots = 128 * sj + p0 + rank
        assert slots.max() < SPAD and rank.max() < KB
        Mb = np.zeros((I, SPAD), np.float32)
        tok2slot = dict(zip(u.tolist(), slots.tolist()))
        slot_of_pos = np.array([tok2slot[t] for t in tokens[b].tolist()])
        Mb[np.arange(I), slot_of_pos] = 1.0
        Pb = np.zeros((SPAD, CHUNK), np.float32)
        Pb[slots, u % CHUNK] = 1.0
        Ms.append(Mb)
        Ps.append(Pb)
    return KB, SPAD, Ms, Ps


def build_program(KB: int, SPAD: int, sw: float, sb: float, pgb: float,
                  nchunk: int = NCHUNK, do_scat: bool = True, stage: int = 99):
    nc = bacc.Bacc()
    heads_d = nc.dram_tensor("heads", (H, TC, I), F32, kind="ExternalInput")
    enc_d = nc.dram_tensor("enc", (I, D), F32, kind="ExternalInput")
    dec_d = nc.dram_tensor("dec", (TC, D), F32, kind="ExternalInput")
    tar_d = nc.dram_tensor("tar", (TC, D), F32, kind="ExternalInput")
    gen_d = nc.dram_tensor("gen", (TC, V), F32, kind="ExternalInput")
    m_d = nc.dram_tensor("m_mat", (I, SPAD), F32R, kind="ExternalInput")
    p_d = nc.dram_tensor("p_mat", (SPAD, CHUNK), F32R, kind="ExternalInput")
    w_d = nc.dram_tensor("wrep", (3, TC, D), F32, kind="ExternalInput")
    id8_d = nc.dram_tensor("id8", (128, 128), F32, kind="ExternalInput")
    fin_d = nc.dram_tensor("fin", (TC, V), F32, kind="ExternalOutput")
    ptr_d = nc.dram_tensor("ptr", (TC, V), F32, kind="ExternalOutput")
    pg_d = nc.dram_tensor("pg", (TC, 1), F32, kind="ExternalOutput")

    NSJ = SPAD // 128

    with tile.TileContext(nc) as tc:
        with (
            tc.tile_pool(name="const", bufs=1) as cpool,
            tc.tile_pool(name="io_in", bufs=8) as ipool,
        ):
          with (
            tc.tile_pool(name="hload", bufs=1) as hpool,
            tc.tile_pool(name="mload", bufs=1) as mpool,
            tc.tile_pool(name="work", bufs=2) as wpool,
            tc.tile_pool(name="ps2", bufs=2, space="PSUM") as ps2,
            tc.tile_pool(name="psctx", bufs=1, space="PSUM") as psctx,
          ):
            # ---------- constant / prologue loads ----------
            id8 = cpool.tile([128, 128], F32, tag="id8")
            nc.sync.dma_start(id8[:], id8_d[:])
            enc_t = []
            for j in range(4):
                e = cpool.tile([128, D], F32, tag=f"enc{j}")
                nc.sync.dma_start(e[:], enc_d[j * 128:(j + 1) * 128, :])
                enc_t.append(e)
            m_t = []
            for j in range(4):
                m = mpool.tile([128, SPAD], F32R, tag=f"m{j}")
                nc.sync.dma_start(m[:], m_d[j * 128:(j + 1) * 128, :])
                m_t.append(m)
            p_t = []
            for sj in range(NSJ):
                p = cpool.tile([128, CHUNK], F32R, tag=f"p{sj}")
                nc.sync.dma_start(p[:], p_d[sj * 128:(sj + 1) * 128, :])
                p_t.append(p)
            wr = []
            for k in range(3):
                w = cpool.tile([TC, D], F32, tag=f"w{k}")
                nc.sync.dma_start(w[:], w_d[k])
                wr.append(w)
            dec_t = cpool.tile([TC, D], F32, tag="dec")
            nc.sync.dma_start(dec_t[:], dec_d[:])
            tar_t = cpool.tile([TC, D], F32, tag="tar")
            nc.sync.dma_start(tar_t[:], tar_d[:])
            ones = cpool.tile([128, 1], F32, tag="ones")
            nc.vector.memset(ones[:], 1.0)
            vbias = cpool.tile([128, 1], F32, tag="vbias")
            nc.vector.memset(vbias[:], float(V - SPAD))
            pgb_t = cpool.tile([128, 1], F32, tag="pgb")
            nc.vector.memset(pgb_t[:], pgb)
            npgb_t = cpool.tile([128, 1], F32, tag="npgb")
            nc.vector.memset(npgb_t[:], -pgb)

            # ---------- attn head-mean + transpose (via Id/8 matmul) ----------
            hts = []
            for h in range(H):
                ht = hpool.tile([TC, I], F32, tag=f"h{h}")
                nc.sync.dma_start(ht[:], heads_d[h])
                hts.append(ht)
            # pairwise in-place tree sum of the 8 heads
            stride = 1
            while stride < H:
                for k in range(0, H, 2 * stride):
                    nc.vector.tensor_add(hts[k][:], hts[k][:],
                                         hts[k + stride][:])
                stride *= 2
            ssum = hts[0]  # (t, i) sum of heads
            if stage == 1:
                nc.sync.dma_start(pg_d[:], ssum[:, 0:1])
                return nc

            aT = []   # attn.T / 8 tiles: (i-part 128, t 128), fp32 for ctx
            aTr = []  # f32r-rounded copies for the f32r g matmuls
            for j in range(4):
                ps = ps2.tile([128, 128], F32, tag="psA")
                nc.tensor.matmul(ps[:], lhsT=ssum[:, j * 128:(j + 1) * 128],
                                 rhs=id8[:], start=True, stop=True)
                a = cpool.tile([128, 128], F32, tag=f"aT{j}")
                nc.vector.tensor_copy(a[:], ps[:])
                aT.append(a)
                ar = cpool.tile([128, 128], F32R, tag=f"aTr{j}")
                nc.vector.tensor_copy(ar[:], ps[:])
                aTr.append(ar)
            if stage == 2:
                nc.sync.dma_start(pg_d[:], aT[0][:, 0:1])
                return nc

            # ---------- context = attn @ enc  (t-part, d) ----------
            ctxps = psctx.tile([128, D], F32, tag="ctx")
            for j in range(4):
                nc.tensor.matmul(ctxps[:], lhsT=aT[j][:], rhs=enc_t[j][:],
                                 start=(j == 0), stop=(j == 3))

            # ---------- gT = M.T @ attn.T  (slot-part, t) ----------
            gt = []
            eg = []
            for sj in range(NSJ):
                gps = ps2.tile([128, 128], F32, tag="psA")
                for j in range(4):
                    nc.tensor.matmul(
                        gps[:],
                        lhsT=m_t[j][:, sj * 128:(sj + 1) * 128],
                        rhs=aTr[j][:],
                        start=(j == 0), stop=(j == 3))
                g = cpool.tile([128, 128], F32R, tag=f"gt{sj}")
                nc.vector.tensor_copy(g[:], gps[:])
                gt.append(g)
                e = mpool.tile([128, 128], F32, tag=f"eg{sj}")
                nc.scalar.activation(e[:], gps[:], AF.Exp)
                eg.append(e)
            if stage == 3:
                nc.sync.dma_start(pg_d[:], gt[0][:, 0:1])
                return nc

            # ---------- lse = log((V - SPAD) + sum_s exp(gT)) per t ----------
            seps = ps2.tile([128, 1], F32, tag="seps")
            for sj in range(NSJ):
                nc.tensor.matmul(seps[:], lhsT=eg[sj][:], rhs=ones[:],
                                 start=(sj == 0), stop=(sj == NSJ - 1))
            lse = cpool.tile([128, 1], F32, tag="lse")
            nc.scalar.activation(lse[:], seps[:], AF.Ln, bias=vbias[:])
            if stage == 4:
                nc.sync.dma_start(pg_d[:], lse[:])
                return nc

            # ---------- p_gen ----------
            zc = cpool.tile([128, 1], F32, tag="zc")
            zd = cpool.tile([128, 1], F32, tag="zd")
            zt = cpool.tile([128, 1], F32, tag="zt")
            for src, wk, acc in ((ctxps, wr[0], zc), (dec_t, wr[1], zd),
                                 (tar_t, wr[2], zt)):
                scr = wpool.tile([TC, D], F32, tag="pscr")
                nc.vector.tensor_mul(scr[:], src[:], wk[:])
                nc.vector.tensor_reduce(acc[:], scr[:],
                                        axis=mybir.AxisListType.X, op=ALU.add)
            z1 = cpool.tile([128, 1], F32, tag="z1")
            nc.vector.tensor_add(z1[:], zc[:], zd[:])
            z = cpool.tile([128, 1], F32, tag="z")
            nc.vector.tensor_add(z[:], z1[:], zt[:])
            pg = cpool.tile([128, 1], F32, tag="pg")
            nc.scalar.activation(pg[:], z[:], AF.Sigmoid, bias=pgb_t[:])
            q = cpool.tile([128, 1], F32, tag="q")
            nc.scalar.activation(q[:], z[:], AF.Sigmoid, scale=-1.0, bias=npgb_t[:])
            nc.sync.dma_start(pg_d[:], pg[:])

            # r2 = sb - sw*lse ; r = q*r2 ; qsw = q*sw
            r2 = cpool.tile([128, 1], F32, tag="r2")
            nc.scalar.activation(r2[:], lse[:], AF.Copy, scale=-sw, bias=sb)
            r = cpool.tile([128, 1], F32, tag="r")
            nc.vector.tensor_mul(r[:], q[:], r2[:])
            qsw = cpool.tile([128, 1], F32, tag="qsw")
            nc.vector.tensor_scalar_mul(qsw[:], q[:], sw)

          # ---------- streamed vocab chunks (prologue pools closed) ----------
          with (
            tc.tile_pool(name="io_out", bufs=6) as opool,
            tc.tile_pool(name="psscat", bufs=4, space="PSUM") as psscat,
          ):
            for n in range(nchunk):
                w = CHUNK if n < NCHUNK - 1 else LASTW
                off = n * CHUNK
                sj, p0 = _chunk_slot_base(n, KB)

                gent = ipool.tile([TC, CHUNK], F32, tag="gen")
                nc.sync.dma_start(gent[:, :w], gen_d[:, off:off + w])

                ptrt = opool.tile([TC, CHUNK], F32, tag="ptr")
                fint = opool.tile([TC, CHUNK], F32, tag="fin")
                if do_scat:
                    scat = psscat.tile([128, CHUNK], F32, tag="scat")
                    for c0 in range(0, w, MMW):
                        cw = min(MMW, w - c0)
                        nc.tensor.matmul(
                            scat[:, c0:c0 + cw],
                            lhsT=gt[sj][p0:p0 + KB, :],
                            rhs=p_t[sj][p0:p0 + KB, c0:c0 + cw],
                            start=True, stop=True)
                    # ptr = sw*scat + r2 ; tmp = qsw*scat + r ; fin = pg*gen+tmp
                    nc.vector.tensor_scalar(
                        ptrt[:, :w], scat[:, :w], sw, r2[:],
                        op0=ALU.mult, op1=ALU.add)
                    tmpt = opool.tile([TC, CHUNK], F32, tag="tmp")
                    nc.scalar.activation(tmpt[:, :w], scat[:, :w], AF.Identity,
                                         scale=qsw[:], bias=r[:])
                    nc.vector.scalar_tensor_tensor(
                        fint[:, :w], in0=gent[:, :w], scalar=pg[:],
                        in1=tmpt[:, :w], op0=ALU.mult, op1=ALU.add)
                else:
                    # background only: ptr = r2 ; fin = pg*gen + r
                    nc.scalar.activation(ptrt[:, :w], gent[:, :w], AF.Identity,
                                         scale=0.0, bias=r2[:])
                    nc.scalar.activation(fint[:, :w], gent[:, :w], AF.Identity,
                                         scale=pg[:], bias=r[:])

                nc.gpsimd.dma_start(fin_d[:, off:off + w], fint[:, :w])
                nc.gpsimd.dma_start(ptr_d[:, off:off + w], ptrt[:, :w])
    return nc


def _in_maps(inputs, Ms, Ps):
    heads = np.asarray(inputs["attn_heads"], np.float32)
    enc = np.asarray(inputs["enc_output"], np.float32)
    dec = np.asarray(inputs["dec_state"], np.float32)
    tar = np.asarray(inputs["tar_embedded"], np.float32)
    gen = np.asarray(inputs["generator_output"], np.float32)
    wfull = np.asarray(inputs["p_gen_w"], np.float32).reshape(3, D)
    wrep = np.broadcast_to(wfull[:, None, :], (3, TC, D)).copy()
    id8 = (np.eye(128, dtype=np.float32) / H).copy()
    maps = []
    for c in range(NCORES):
        b, t0 = c // CORES_PER_B, (c % CORES_PER_B) * TC
        maps.append({
            "heads": np.ascontiguousarray(heads[b, :, t0:t0 + TC, :]),
            "enc": np.ascontiguousarray(enc[b]),
            "dec": np.ascontiguousarray(dec[b, t0:t0 + TC]),
            "tar": np.ascontiguousarray(tar[b, t0:t0 + TC]),
            "gen": np.ascontiguousarray(gen[b, t0:t0 + TC]),
            "m_mat": Ms[b],
            "p_mat": Ps[b],
            "wrep": wrep,
            "id8": id8,
        })
    return maps


def _run(inputs, trace=False):
    tokens = np.asarray(inputs["inp_tokens"])
    KB, SPAD, Ms, Ps = build_schedule(tokens)
    sw = float(np.asarray(inputs["pointer_scale_w"]))
    sb = float(np.asarray(inputs["pointer_scale_b"]))
    pgb = float(np.asarray(inputs["p_gen_b"]).reshape(-1)[0])
    nc = build_program(KB, SPAD, sw, sb, pgb)
    nc.finalize()
    maps = _in_maps(inputs, Ms, Ps)
    res = run_bass_kernel_spmd(nc, maps, list(range(NCORES)), trace=trace)
    fin = np.empty((B, T, V), np.float32)
    ptr = np.empty((B, T, V), np.float32)
    pgo = np.empty((B, T), np.float32)
    for c in range(NCORES):
        b, t0 = c // CORES_PER_B, (c % CORES_PER_B) * TC
        fin[b, t0:t0 + TC] = res.results[c]["fin"]
        ptr[b, t0:t0 + TC] = res.results[c]["ptr"]
        pgo[b, t0:t0 + TC] = res.results[c]["pg"][:, 0]
    return (fin, ptr, pgo), res


def kernel(**inputs):
    out, _ = _run(inputs, trace=False)
    return out


def _install_profile_hook():
    """Provide antenv.axon_hooks (missing in this container) so
    run_bass_kernel_spmd(trace=True) can reach libaxon's NRT profiler."""
    import contextlib
    import ctypes
    import sys
    import types
    try:
        import antenv.axon_hooks  # noqa: F401
        return
    except ImportError:
        pass

    so_path = "/opt/axon/libaxon_pjrt.so"
    lib = ctypes.CDLL(so_path)
    hook = None
    if hasattr(lib, "axon_start_nrt_profile"):
        lib.axon_start_nrt_profile.argtypes = [
            ctypes.POINTER(ctypes.c_int64), ctypes.c_size_t]
        lib.axon_start_nrt_profile.restype = ctypes.c_int64
        lib.axon_stop_nrt_profile.argtypes = [ctypes.c_char_p]
        lib.axon_stop_nrt_profile.restype = ctypes.c_int64

        @contextlib.contextmanager
        def hook(output_dir, device_ids):
            import jax
            jax.devices()
            if device_ids:
                ids = (ctypes.c_int64 * len(device_ids))(*device_ids)
                rc = lib.axon_start_nrt_profile(ids, len(device_ids))
            else:
                rc = lib.axon_start_nrt_profile(None, 0)
            if rc != 0:
                raise RuntimeError(f"axon_start_nrt_profile rc={rc}")
            try:
                yield
            finally:
                n = lib.axon_stop_nrt_profile(str(output_dir).encode())
                print(f"profile: {n} ntff file(s) -> {output_dir}",
                      file=sys.stderr)

    mod = types.ModuleType("antenv.axon_hooks")
    mod.get_axon_ntff_profile_hook = lambda: hook
    mod.set_axon_ntff_profile_hook = lambda h: None
    sys.modules["antenv.axon_hooks"] = mod


def kernel_profiled(**inputs):
    _install_profile_hook()
    import concourse.bass_utils as bu
    bu.upload_artifacts = lambda tmpdir: str(tmpdir)  # no bucket in container
    out, res = _run(inputs, trace=True)
    return out, res


# revision 46
# speedup vs baseline: 5.2678x; 1.1246x over previous
"""PointerNet fused kernel for Trainium2 (8 NeuronCores, Bass/Tile).

Problem (B=4, T=256, I=512, H=8, V=32000, D=512):
    attn    = mean_h(attn_heads)                          (B,T,I)
    context = attn @ enc_output                           (B,T,D)
    p_gen   = sigmoid([context|dec|tar] @ w + b)          (B,T,1)
    pointer = log_softmax(scatter_add(attn -> vocab))     (B,T,V)
    pointer = sw * pointer + sb
    final   = p_gen * generator_output + (1-p_gen) * pointer
    returns (final, pointer, squeeze(p_gen))

Key observation: the scattered tensor is zero outside the <=512 token
columns per batch, so log_softmax rows are a per-row constant
(-lse, from a 512-slot logsumexp) except at token columns.  We never
materialize the dense scatter in DRAM:

  * Host builds, per batch, a duplicate-merge one-hot M (I x SPAD) that
    maps input positions to distinct-token "slots" (sorted by token id,
    grouped so chunk n of the vocab owns slot range [KB*n, KB*n+KB)),
    and a placement one-hot P (SPAD x 512) mapping slot -> column-within-
    chunk.  KB is a divisor of 128 so slot ranges never straddle SBUF
    partition tiles, giving a single uniform SPMD program for all cores.
  * Device computes gT = M.T @ attn.T (slots x T) once, then for each
    512-wide vocab chunk does ONE tiny matmul (K=KB) producing the dense
    scatter chunk directly in PSUM, fused with the log_softmax affine and
    the p_gen blend while streaming generator_output through SBUF.

Per-core traffic ~58MB (read gen 16.4MB + heads 2.1 + misc, write
fin/ptr 32.8MB) -> memory-roofline bound.

Sharding: (B,T) rows flattened to 1024 rows, 128 rows/core; core c
handles batch c//2, t-range (c%2)*128..+128.  No collectives needed.
"""

import numpy as np

import concourse.bass as bass
import concourse.tile as tile
from concourse import bacc, mybir
from concourse.bass_utils import run_bass_kernel_spmd

F32 = mybir.dt.float32
F32R = mybir.dt.float32r  # single-pass fp32 matmul mode (4x faster at N>=256)
AF = mybir.ActivationFunctionType
ALU = mybir.AluOpType

B, T, I, H, V, D = 4, 256, 512, 8, 32000, 512
NCORES = 8
CORES_PER_B = NCORES // B          # 2
TC = T // CORES_PER_B              # 128 rows per core
CHUNK = 1024
NCHUNK = (V + CHUNK - 1) // CHUNK  # 32 (last chunk width 256)
LASTW = V - (NCHUNK - 1) * CHUNK
MMW = 512                          # max matmul moving free dim / PSUM bank


def _pick_kb(max_per_chunk: int) -> int:
    # PE matmul operands must start at partition 0/32/64 (not 96) -> pack
    # chunks per 128-partition tile at those bases only.
    for kb in (32, 64, 128):
        if kb >= max_per_chunk:
            return kb
    raise ValueError(f"too many distinct tokens in one vocab chunk: {max_per_chunk}")


def _chunk_slot_base(n: int, KB: int) -> tuple[int, int]:
    """Return (tile index sj, partition base p0) for vocab chunk n."""
    cpt = {32: 3, 64: 2, 128: 1}[KB]  # chunks per 128-partition tile
    return n // cpt, KB * (n % cpt)


def build_schedule(tokens: np.ndarray):
    """tokens: (B, I) int. Returns (KB, SPAD, M[B], P[B])."""
    uniq = [np.unique(tokens[b]) for b in range(B)]
    mx = 1
    for u in uniq:
        cnt = np.bincount(u // CHUNK, minlength=NCHUNK)
        mx = max(mx, int(cnt.max()))
    KB = _pick_kb(mx)
    cpt = {32: 3, 64: 2, 128: 1}[KB]
    SPAD = ((NCHUNK + cpt - 1) // cpt) * 128
    Ms, Ps = [], []
    for b in range(B):
        u = uniq[b]
        chunks = u // CHUNK
        rank = np.arange(len(u)) - np.searchsorted(chunks, chunks, side="left")
        sj, p0 = np.vectorize(lambda n: _chunk_slot_base(n, KB))(chunks)
        slots = 128 * sj + p0 + rank
        assert slots.max() < SPAD and rank.max() < KB
        Mb = np.zeros((I, SPAD), np.float32)
        tok2slot = dict(zip(u.tolist(), slots.tolist()))
        slot_of_pos = np.array([tok2slot[t] for t in tokens[b].tolist()])
        Mb[np.arange(I), slot_of_pos] = 1.0
        Pb = np.zeros((SPAD, CHUNK), np.float32)
        Pb[slots, u % CHUNK] = 1.0
        Ms.append(Mb)
        Ps.append(Pb)
    return KB, SPAD, Ms, Ps


def build_program(KB: int, SPAD: int, sw: float, sb: float, pgb: float,
                  nchunk: int = NCHUNK, do_scat: bool = True, stage: int = 99):
    nc = bacc.Bacc()
    heads_d = nc.dram_tensor("heads", (H, TC, I), F32, kind="ExternalInput")
    enc_d = nc.dram_tensor("enc", (I, D), F32, kind="ExternalInput")
    dec_d = nc.dram_tensor("dec", (TC, D), F32, kind="ExternalInput")
    tar_d = nc.dram_tensor("tar", (TC, D), F32, kind="ExternalInput")
    gen_d = nc.dram_tensor("gen", (TC, V), F32, kind="ExternalInput")
    m_d = nc.dram_tensor("m_mat", (I, SPAD), F32R, kind="ExternalInput")
    p_d = nc.dram_tensor("p_mat", (SPAD, CHUNK), F32R, kind="ExternalInput")
    w_d = nc.dram_tensor("wrep", (3, TC, D), F32, kind="ExternalInput")
    id8_d = nc.dram_tensor("id8", (128, 128), F32, kind="ExternalInput")
    fin_d = nc.dram_tensor("fin", (TC, V), F32, kind="ExternalOutput")
    ptr_d = nc.dram_tensor("ptr", (TC, V), F32, kind="ExternalOutput")
    pg_d = nc.dram_tensor("pg", (TC, 1), F32, kind="ExternalOutput")

    NSJ = SPAD // 128

    with tile.TileContext(nc) as tc:
        with (
            tc.tile_pool(name="const", bufs=1) as cpool,
            tc.tile_pool(name="io_in", bufs=8) as ipool,
        ):
          with (
            tc.tile_pool(name="hload", bufs=1) as hpool,
            tc.tile_pool(name="mload", bufs=1) as mpool,
            tc.tile_pool(name="work", bufs=2) as wpool,
            tc.tile_pool(name="ps2", bufs=2, space="PSUM") as ps2,
            tc.tile_pool(name="psctx", bufs=1, space="PSUM") as psctx,
          ):
            # ---------- prologue loads: heads FIRST (critical path) ----------
            hts = []
            for h in range(H):
                ht = hpool.tile([TC, I], F32, tag=f"h{h}")
                nc.sync.dma_start(ht[:], heads_d[h])
                hts.append(ht)
            id8 = cpool.tile([128, 128], F32, tag="id8")
            nc.sync.dma_start(id8[:], id8_d[:])
            enc_t = []
            for j in range(4):
                e = cpool.tile([128, D], F32, tag=f"enc{j}")
                nc.sync.dma_start(e[:], enc_d[j * 128:(j + 1) * 128, :])
                enc_t.append(e)
            wr = []
            for k in range(3):
                w = cpool.tile([TC, D], F32, tag=f"w{k}")
                nc.sync.dma_start(w[:], w_d[k])
                wr.append(w)
            dec_t = cpool.tile([TC, D], F32, tag="dec")
            nc.sync.dma_start(dec_t[:], dec_d[:])
            tar_t = cpool.tile([TC, D], F32, tag="tar")
            nc.sync.dma_start(tar_t[:], tar_d[:])
            m_t = []
            for j in range(4):
                m = mpool.tile([128, SPAD], F32R, tag=f"m{j}")
                nc.sync.dma_start(m[:], m_d[j * 128:(j + 1) * 128, :])
                m_t.append(m)
            p_t = []
            for sj in range(NSJ):
                p = cpool.tile([128, CHUNK], F32R, tag=f"p{sj}")
                nc.sync.dma_start(p[:], p_d[sj * 128:(sj + 1) * 128, :])
                p_t.append(p)
            ones = cpool.tile([128, 1], F32, tag="ones")
            nc.vector.memset(ones[:], 1.0)
            vbias = cpool.tile([128, 1], F32, tag="vbias")
            nc.vector.memset(vbias[:], float(V - SPAD))
            pgb_t = cpool.tile([128, 1], F32, tag="pgb")
            nc.vector.memset(pgb_t[:], pgb)
            npgb_t = cpool.tile([128, 1], F32, tag="npgb")
            nc.vector.memset(npgb_t[:], -pgb)

            # ---------- attn head-mean + transpose (via Id/8 matmul) ----------
            # pairwise in-place tree sum of the 8 heads
            stride = 1
            while stride < H:
                for k in range(0, H, 2 * stride):
                    nc.vector.tensor_add(hts[k][:], hts[k][:],
                                         hts[k + stride][:])
                stride *= 2
            ssum = hts[0]  # (t, i) sum of heads
            if stage == 1:
                nc.sync.dma_start(pg_d[:], ssum[:, 0:1])
                return nc

            aT = []   # attn.T / 8 tiles: (i-part 128, t 128), fp32 for ctx
            aTr = []  # f32r-rounded copies for the f32r g matmuls
            for j in range(4):
                ps = ps2.tile([128, 128], F32, tag="psA")
                nc.tensor.matmul(ps[:], lhsT=ssum[:, j * 128:(j + 1) * 128],
                                 rhs=id8[:], start=True, stop=True)
                a = cpool.tile([128, 128], F32, tag=f"aT{j}")
                nc.vector.tensor_copy(a[:], ps[:])
                aT.append(a)
                ar = cpool.tile([128, 128], F32R, tag=f"aTr{j}")
                nc.vector.tensor_copy(ar[:], ps[:])
                aTr.append(ar)
            if stage == 2:
                nc.sync.dma_start(pg_d[:], aT[0][:, 0:1])
                return nc

            # ---------- context = attn @ enc  (t-part, d) ----------
            ctxps = psctx.tile([128, D], F32, tag="ctx")
            for j in range(4):
                nc.tensor.matmul(ctxps[:], lhsT=aT[j][:], rhs=enc_t[j][:],
                                 start=(j == 0), stop=(j == 3))

            # ---------- gT = M.T @ attn.T  (slot-part, t) ----------
            gt = []
            eg = []
            for sj in range(NSJ):
                gps = ps2.tile([128, 128], F32, tag="psA")
                for j in range(4):
                    nc.tensor.matmul(
                        gps[:],
                        lhsT=m_t[j][:, sj * 128:(sj + 1) * 128],
                        rhs=aTr[j][:],
                        start=(j == 0), stop=(j == 3))
                g = cpool.tile([128, 128], F32R, tag=f"gt{sj}")
                nc.vector.tensor_copy(g[:], gps[:])
                gt.append(g)
                e = mpool.tile([128, 128], F32, tag=f"eg{sj}")
                nc.scalar.activation(e[:], gps[:], AF.Exp)
                eg.append(e)
            if stage == 3:
                nc.sync.dma_start(pg_d[:], gt[0][:, 0:1])
                return nc

            # ---------- lse = log((V - SPAD) + sum_s exp(gT)) per t ----------
            seps = ps2.tile([128, 1], F32, tag="seps")
            for sj in range(NSJ):
                nc.tensor.matmul(seps[:], lhsT=eg[sj][:], rhs=ones[:],
                                 start=(sj == 0), stop=(sj == NSJ - 1))
            lse = cpool.tile([128, 1], F32, tag="lse")
            nc.scalar.activation(lse[:], seps[:], AF.Ln, bias=vbias[:])
            if stage == 4:
                nc.sync.dma_start(pg_d[:], lse[:])
                return nc

            # ---------- p_gen ----------
            zc = cpool.tile([128, 1], F32, tag="zc")
            zd = cpool.tile([128, 1], F32, tag="zd")
            zt = cpool.tile([128, 1], F32, tag="zt")
            for src, wk, acc in ((ctxps, wr[0], zc), (dec_t, wr[1], zd),
                                 (tar_t, wr[2], zt)):
                scr = wpool.tile([TC, D], F32, tag="pscr")
                nc.vector.tensor_mul(scr[:], src[:], wk[:])
                nc.vector.tensor_reduce(acc[:], scr[:],
                                        axis=mybir.AxisListType.X, op=ALU.add)
            z1 = cpool.tile([128, 1], F32, tag="z1")
            nc.vector.tensor_add(z1[:], zc[:], zd[:])
            z = cpool.tile([128, 1], F32, tag="z")
            nc.vector.tensor_add(z[:], z1[:], zt[:])
            pg = cpool.tile([128, 1], F32, tag="pg")
            nc.scalar.activation(pg[:], z[:], AF.Sigmoid, bias=pgb_t[:])
            q = cpool.tile([128, 1], F32, tag="q")
            nc.scalar.activation(q[:], z[:], AF.Sigmoid, scale=-1.0, bias=npgb_t[:])
            nc.sync.dma_start(pg_d[:], pg[:])

            # r2 = sb - sw*lse ; r = q*r2 ; qsw = q*sw
            r2 = cpool.tile([128, 1], F32, tag="r2")
            nc.scalar.activation(r2[:], lse[:], AF.Copy, scale=-sw, bias=sb)
            r = cpool.tile([128, 1], F32, tag="r")
            nc.vector.tensor_mul(r[:], q[:], r2[:])
            qsw = cpool.tile([128, 1], F32, tag="qsw")
            nc.vector.tensor_scalar_mul(qsw[:], q[:], sw)

          # ---------- streamed vocab chunks (prologue pools closed) ----------
          with (
            tc.tile_pool(name="io_out", bufs=6) as opool,
            tc.tile_pool(name="psscat", bufs=4, space="PSUM") as psscat,
          ):
            for n in range(nchunk):
                w = CHUNK if n < NCHUNK - 1 else LASTW
                off = n * CHUNK
                sj, p0 = _chunk_slot_base(n, KB)

                gent = ipool.tile([TC, CHUNK], F32, tag="gen")
                nc.sync.dma_start(gent[:, :w], gen_d[:, off:off + w])

                ptrt = opool.tile([TC, CHUNK], F32, tag="ptr")
                fint = opool.tile([TC, CHUNK], F32, tag="fin")
                if do_scat:
                    scat = psscat.tile([128, CHUNK], F32, tag="scat")
                    for c0 in range(0, w, MMW):
                        cw = min(MMW, w - c0)
                        nc.tensor.matmul(
                            scat[:, c0:c0 + cw],
                            lhsT=gt[sj][p0:p0 + KB, :],
                            rhs=p_t[sj][p0:p0 + KB, c0:c0 + cw],
                            start=True, stop=True)
                    # ptr = sw*scat + r2 ; tmp = qsw*scat + r ; fin = pg*gen+tmp
                    nc.vector.tensor_scalar(
                        ptrt[:, :w], scat[:, :w], sw, r2[:],
                        op0=ALU.mult, op1=ALU.add)
                    tmpt = opool.tile([TC, CHUNK], F32, tag="tmp")
                    nc.scalar.activation(tmpt[:, :w], scat[:, :w], AF.Identity,
                                         scale=qsw[:], bias=r[:])
                    nc.vector.scalar_tensor_tensor(
                        fint[:, :w], in0=gent[:, :w], scalar=pg[:],
                        in1=tmpt[:, :w], op0=ALU.mult, op1=ALU.add)
                else:
                    # background only: ptr = r2 ; fin = pg*gen + r
                    nc.scalar.activation(ptrt[:, :w], gent[:, :w], AF.Identity,
                                         scale=0.0, bias=r2[:])
                    nc.scalar.activation(fint[:, :w], gent[:, :w], AF.Identity,
                                         scale=pg[:], bias=r[:])

                nc.gpsimd.dma_start(fin_d[:, off:off + w], fint[:, :w])
                nc.gpsimd.dma_start(ptr_d[:, off:off + w], ptrt[:, :w])
    return nc


def _in_maps(inputs, Ms, Ps):
    heads = np.asarray(inputs["attn_heads"], np.float32)
    enc = np.asarray(inputs["enc_output"], np.float32)
    dec = np.asarray(inputs["dec_state"], np.float32)
    tar = np.asarray(inputs["tar_embedded"], np.float32)
    gen = np.asarray(inputs["generator_output"], np.float32)
    wfull = np.asarray(inputs["p_gen_w"], np.float32).reshape(3, D)
    wrep = np.broadcast_to(wfull[:, None, :], (3, TC, D)).copy()
    id8 = (np.eye(128, dtype=np.float32) / H).copy()
    maps = []
    for c in range(NCORES):
        b, t0 = c // CORES_PER_B, (c % CORES_PER_B) * TC
        maps.append({
            "heads": np.ascontiguousarray(heads[b, :, t0:t0 + TC, :]),
            "enc": np.ascontiguousarray(enc[b]),
            "dec": np.ascontiguousarray(dec[b, t0:t0 + TC]),
            "tar": np.ascontiguousarray(tar[b, t0:t0 + TC]),
            "gen": np.ascontiguousarray(gen[b, t0:t0 + TC]),
            "m_mat": Ms[b],
            "p_mat": Ps[b],
            "wrep": wrep,
            "id8": id8,
        })
    return maps


def _run(inputs, trace=False):
    tokens = np.asarray(inputs["inp_tokens"])
    KB, SPAD, Ms, Ps = build_schedule(tokens)
    sw = float(np.asarray(inputs["pointer_scale_w"]))
    sb = float(np.asarray(inputs["pointer_scale_b"]))
    pgb = float(np.asarray(inputs["p_gen_b"]).reshape(-1)[0])
    nc = build_program(KB, SPAD, sw, sb, pgb)
    nc.finalize()
    maps = _in_maps(inputs, Ms, Ps)
    res = run_bass_kernel_spmd(nc, maps, list(range(NCORES)), trace=trace)
    fin = np.empty((B, T, V), np.float32)
    ptr = np.empty((B, T, V), np.float32)
    pgo = np.empty((B, T), np.float32)
    for c in range(NCORES):
        b, t0 = c // CORES_PER_B, (c % CORES_PER_B) * TC
        fin[b, t0:t0 + TC] = res.results[c]["fin"]
        ptr[b, t0:t0 + TC] = res.results[c]["ptr"]
        pgo[b, t0:t0 + TC] = res.results[c]["pg"][:, 0]
    return (fin, ptr, pgo), res


def kernel(**inputs):
    out, _ = _run(inputs, trace=False)
    return out


def _install_profile_hook():
    """Provide antenv.axon_hooks (missing in this container) so
    run_bass_kernel_spmd(trace=True) can reach libaxon's NRT profiler."""
    import contextlib
    import ctypes
    import sys
    import types
    try:
        import antenv.axon_hooks  # noqa: F401
        return
    except ImportError:
        pass

    so_path = "/opt/axon/libaxon_pjrt.so"
    lib = ctypes.CDLL(so_path)
    hook = None
    if hasattr(lib, "axon_start_nrt_profile"):
        lib.axon_start_nrt_profile.argtypes = [
            ctypes.POINTER(ctypes.c_int64), ctypes.c_size_t]
        lib.axon_start_nrt_profile.restype = ctypes.c_int64
        lib.axon_stop_nrt_profile.argtypes = [ctypes.c_char_p]
        lib.axon_stop_nrt_profile.restype = ctypes.c_int64

        @contextlib.contextmanager
        def hook(output_dir, device_ids):
            import jax
            jax.devices()
            if device_ids:
                ids = (ctypes.c_int64 * len(device_ids))(*device_ids)
                rc = lib.axon_start_nrt_profile(ids, len(device_ids))
            else:
                rc = lib.axon_start_nrt_profile(None, 0)
            if rc != 0:
                raise RuntimeError(f"axon_start_nrt_profile rc={rc}")
            try:
                yield
            finally:
                n = lib.axon_stop_nrt_profile(str(output_dir).encode())
                print(f"profile: {n} ntff file(s) -> {output_dir}",
                      file=sys.stderr)

    mod = types.ModuleType("antenv.axon_hooks")
    mod.get_axon_ntff_profile_hook = lambda: hook
    mod.set_axon_ntff_profile_hook = lambda h: None
    sys.modules["antenv.axon_hooks"] = mod


def kernel_profiled(**inputs):
    _install_profile_hook()
    import concourse.bass_utils as bu
    bu.upload_artifacts = lambda tmpdir: str(tmpdir)  # no bucket in container
    out, res = _run(inputs, trace=True)
    return out, res
